# revision 1
# baseline (speedup 1.0000x reference)
"""Farthest-point-sampling (npoint=2) Bass kernel for Trainium2.

Problem: xyz [1, 64, 3, 262144] fp32 -> indices [64, 2] (int64 on host).
Per batch b:
  idx0 = argmax_n y[n]            (y = coord plane 1)
  c    = (x,y,z)[idx0]
  idx1 = argmax_n ((x-cx)^2 + (y-cy)^2 + (z-cz)^2)
argmax = first occurrence on ties (jnp.argmax semantics).

Sharding: data-parallel over batch; 8 NeuronCores x 8 batches each.

Per-core structure (planes viewed as [128, 2048] fp32):
  Phase 0 (all 8 batches): DMA y plane; VectorE Max8 + MaxIndex ->
    per-partition (top-8, cols); stash col-0 max and (N - global_idx)
    candidate into defer tiles.
  Y finale (batched): PE-transpose the 8 batches' [128,1] pairs into
    rows, then reduce/select tiny ops produce idx0 per batch.
    candidate = N - global_idx so the max picks the smallest index among
    equal maxima (first-occurrence tie semantics).
  Per batch phase B: PE ones-matmul broadcasts idx0 -> [3,1]; offsets
    stt; indirect-DMA gather of centroid [3,1]; PE transpose + ScalarE
    negate -> [1,3]; PE ones-matmul broadcast -> [128,3] bias tile;
    paired x+z DMA; ScalarE Square(v + (-c)) x3; GpSimd adds
    s1 = sqx+sqy, s2 = s1+sqz; VectorE Max8 + MaxIndex on s2; stash.
  Dist finale (batched): same as Y finale -> idx1 per batch.
All GPSIMD ops are 'standard'-library (iota, tensor_tensor) or DGE —
no mid-kernel ucode library swaps.
"""

import numpy as np

import concourse.bacc as bacc
import concourse.bass as bass
import concourse.mybir as mybir
from concourse.masks import make_identity
from concourse.tile import TileContext

B = 64  # full batch
N_CORES = 8
BPC = B // N_CORES  # batches per core
N = 262144
P = 128
COLS = N // P  # 2048
BIGK = float(N)

F32 = mybir.dt.float32
U32 = mybir.dt.uint32
I32 = mybir.dt.int32
AX = mybir.AxisListType.X
OP = mybir.AluOpType
SQUARE = mybir.ActivationFunctionType.Square


def build_nc():
    nc = bacc.Bacc()
    xin = nc.dram_tensor("xyz", [BPC, 3, N], F32, kind="ExternalInput")
    out = nc.dram_tensor("idx", [1, 2 * BPC], I32, kind="ExternalOutput")

    with TileContext(nc) as tc:
        with (
            tc.tile_pool(name="consts", bufs=1) as consts,
            tc.tile_pool(name="ypool", bufs=BPC) as ypool,
            tc.tile_pool(name="big", bufs=2) as big,
            tc.tile_pool(name="small", bufs=4) as small,
            tc.tile_pool(name="acc", bufs=1) as acc,
            tc.tile_pool(name="psb", bufs=2, space="PSUM") as psb,
            tc.tile_pool(name="psf", bufs=1, space="PSUM") as psf,
        ):
            # ---- constants ----
            ident = consts.tile([P, P], F32)
            make_identity(nc, ident)
            ones = consts.tile([1, P], F32)
            nc.vector.memset(ones, 1.0)
            # revb[p] = N - p*COLS ; pbase[c] = c*N   (exact in f32 < 2^24)
            revb_i = consts.tile([P, 1], I32)
            nc.gpsimd.iota(revb_i, pattern=[[0, 1]], base=N, channel_multiplier=-COLS)
            revb_f = consts.tile([P, 1], F32)
            nc.vector.tensor_copy(revb_f, revb_i)
            pbase = consts.tile([3, 1], I32)
            nc.gpsimd.iota(pbase, pattern=[[0, 1]], base=0, channel_multiplier=N)

            out_i = acc.tile([1, 2 * BPC], I32)  # cols 0..7 idx0, 8..15 idx1
            # wide defer tiles: max8/max_index write straight into them
            dYV8 = acc.tile([P, 8 * BPC], F32)
            dYI8 = acc.tile([P, 8 * BPC], U32)
            dDV8 = acc.tile([P, 8 * BPC], F32)
            dDI8 = acc.tile([P, 8 * BPC], U32)

            def col0(t):
                return t.rearrange("p (b k) -> p b k", k=8)[:, :, 0]

            def batched_finale(dv8, di8, out_cols, tagp):
                """dv8/di8: [P, 8*BPC] per-batch top-8 (vals, cols).
                Returns SBUF [1, BPC] f32 of winning global indices; also
                writes them (cast i32) into out_i[:, out_cols]."""
                i8f = small.tile([P, BPC], F32, tag=f"i8f{tagp}")
                nc.vector.tensor_copy(i8f, col0(di8))
                candall = small.tile([P, BPC], F32, tag=f"ca{tagp}")
                nc.vector.tensor_sub(
                    candall, revb_f.to_broadcast([P, BPC]), i8f
                )
                ptv = psf.tile([BPC, 2 * P], F32, tag="ptv")
                nc.tensor.transpose(ptv[0:BPC, 0:P], col0(dv8), ident)
                nc.tensor.transpose(ptv[0:BPC, P : 2 * P], candall, ident)
                rows = small.tile([BPC, 2 * P], F32, tag=f"rows{tagp}")
                nc.vector.tensor_copy(rows, ptv)
                mxs = small.tile([BPC, 1], F32, tag=f"mxs{tagp}")
                nc.vector.tensor_reduce(mxs, rows[:, 0:P], axis=AX, op=OP.max)
                cands = small.tile([BPC, P], F32, tag=f"cands{tagp}")
                nc.vector.scalar_tensor_tensor(
                    out=cands,
                    in0=rows[:, 0:P],
                    scalar=mxs[:, 0:1],
                    in1=rows[:, P : 2 * P],
                    op0=OP.is_equal,
                    op1=OP.mult,
                )
                rs = small.tile([BPC, 1], F32, tag=f"rs{tagp}")
                nc.vector.tensor_reduce(rs, cands, axis=AX, op=OP.max)
                idxs = small.tile([BPC, 1], F32, tag=f"idxs{tagp}")
                nc.vector.tensor_scalar(
                    out=idxs, in0=rs, scalar1=-1.0, scalar2=BIGK,
                    op0=OP.mult, op1=OP.add,
                )
                pti = psf.tile([1, BPC], F32, tag="pti")
                nc.tensor.transpose(pti, idxs, ident[0:BPC, 0:BPC])
                rowi = small.tile([1, BPC], F32, tag=f"rowi{tagp}")
                nc.vector.tensor_copy(rowi, pti)
                nc.scalar.copy(out_i[0:1, out_cols], rowi)
                return rowi

            # ---------- phase 0: y argmax per batch ----------
            tys = []
            for b in range(BPC):
                ty = ypool.tile([P, COLS], F32, tag="ty")
                tys.append(ty)
                nc.sync.dma_start(ty, xin[b, 1].rearrange("(p m) -> p m", p=P))
                nc.vector.max(out=dYV8[:, 8 * b : 8 * b + 8], in_=ty)
                nc.vector.max_index(
                    dYI8[:, 8 * b : 8 * b + 8], dYV8[:, 8 * b : 8 * b + 8], ty
                )

            idx0row = batched_finale(dYV8, dYI8, slice(0, BPC), "y")

            # ---------- phase B per batch ----------
            for b in range(BPC):
                # idx0 -> [3,1] via PE ones-matmul; offsets; gather centroid
                p3 = psb.tile([3, 1], F32, tag="p3")
                nc.tensor.matmul(
                    p3, ones[0:1, 0:3], idx0row[0:1, b : b + 1],
                    start=True, stop=True,
                )
                offs = small.tile([3, 1], U32, tag="offs")
                # offs[c] = idx0 + b*3N + c*N (flat index into xin)
                nc.vector.scalar_tensor_tensor(
                    out=offs, in0=p3, scalar=float(b * 3 * N), in1=pbase,
                    op0=OP.add, op1=OP.add,
                )
                c3 = small.tile([3, 1], F32, tag="c3")
                nc.gpsimd.indirect_dma_start(
                    out=c3,
                    out_offset=None,
                    in_=xin.rearrange("b c n -> (b c n)")[:, None],
                    in_offset=bass.IndirectOffsetOnAxis(ap=offs[0:3, 0:1], axis=0),
                )
                # negate + broadcast to [128,3] bias tile via PE
                pc3 = psb.tile([1, 3], F32, tag="pc3")
                nc.tensor.transpose(pc3, c3, ident[0:3, 0:3])
                negrow = small.tile([1, 3], F32, tag="negrow")
                nc.scalar.mul(negrow, pc3, -1.0)
                pnegc = psb.tile([P, 3], F32, tag="pnegc")
                nc.tensor.matmul(pnegc, ones, negrow, start=True, stop=True)
                negc = small.tile([P, 3], F32, tag="negc")
                nc.vector.tensor_copy(negc, pnegc)

                # x and z planes in one strided DMA: [P, 2, COLS]
                txz = big.tile([P, 2, COLS], F32, tag="txz")
                nc.sync.dma_start(
                    txz,
                    xin[b, 0::2].rearrange("c (p m) -> p c m", p=P),
                )
                sqx = big.tile([P, COLS], F32, tag="sqx")
                nc.scalar.activation(sqx, txz[:, 0], SQUARE, bias=negc[:, 0:1])
                sqy = big.tile([P, COLS], F32, tag="sqy")
                nc.scalar.activation(sqy, tys[b], SQUARE, bias=negc[:, 1:2])
                sqz = big.tile([P, COLS], F32, tag="sqz")
                nc.scalar.activation(sqz, txz[:, 1], SQUARE, bias=negc[:, 2:3])

                # adds split ~75/25 between GpSimd and VectorE
                CS = 1536
                s1 = big.tile([P, COLS], F32, tag="s1")
                nc.gpsimd.tensor_add(s1[:, 0:CS], sqx[:, 0:CS], sqy[:, 0:CS])
                nc.vector.tensor_add(s1[:, CS:], sqx[:, CS:], sqy[:, CS:])
                s2 = big.tile([P, COLS], F32, tag="s2")
                nc.gpsimd.tensor_add(s2[:, 0:CS], s1[:, 0:CS], sqz[:, 0:CS])
                nc.vector.tensor_add(s2[:, CS:], s1[:, CS:], sqz[:, CS:])

                nc.vector.max(out=dDV8[:, 8 * b : 8 * b + 8], in_=s2)
                nc.vector.max_index(
                    dDI8[:, 8 * b : 8 * b + 8], dDV8[:, 8 * b : 8 * b + 8], s2
                )

            batched_finale(dDV8, dDI8, slice(BPC, 2 * BPC), "d")

            nc.sync.dma_start(out[:, :], out_i[:, :])

    nc.compile()
    return nc


_NC_CACHE = None


def _get_nc():
    global _NC_CACHE
    if _NC_CACHE is None:
        _NC_CACHE = build_nc()
    return _NC_CACHE


def kernel(xyz: np.ndarray) -> np.ndarray:
    from concourse.bass_utils import run_bass_kernel_spmd

    assert xyz.shape == (1, B, 3, N), xyz.shape
    xyz = np.ascontiguousarray(xyz, dtype=np.float32)
    nc = _get_nc()
    in_maps = [
        {"xyz": np.ascontiguousarray(xyz[0, k * BPC : (k + 1) * BPC])}
        for k in range(N_CORES)
    ]
    res = run_bass_kernel_spmd(nc, in_maps, core_ids=list(range(N_CORES)))
    # out layout per core: [1, 16] = [idx0 x8 | idx1 x8]
    outs = [res.results[k]["idx"].reshape(2, BPC).T for k in range(N_CORES)]
    return np.concatenate(outs, axis=0).astype(np.int64)



# revision 4
# speedup vs baseline: 1.1687x; 1.1687x over previous
"""Farthest-point-sampling (npoint=2) Bass kernel for Trainium2 — v2.

Problem: xyz [1, 64, 3, 262144] fp32 -> indices [64, 2] (int64 on host).
Per batch b:
  idx0 = argmax_n y[n]            (y = coord plane 1)
  c    = (x,y,z)[idx0]
  idx1 = argmax_n ((x-cx)^2 + (y-cy)^2 + (z-cz)^2)
argmax = first occurrence on ties (jnp.argmax semantics).

Sharding: data-parallel over batch; 8 NeuronCores x 8 batches each.

v2 structure (per core, 8 batches; plane viewed as [128, 2048] fp32):
  * argmax via chunk hierarchy: ONE VectorE grouped tensor_reduce
    [128,(16,128)] -> chunk maxima M [128,16]; a tiny transpose-finale on
    M finds the global max and its (partition, chunk) = pk code with
    first-occurrence tie order; an indirect-DMA gather pulls the winning
    128-wide window back from HBM and a tiny find locates the column.
  * distance tensor: ScalarE squares (v + (-c))^2 — bit-exact vs the
    reference; the x-plane square is written straight into PSUM, PE
    identity-matmuls accumulate the y and z squared planes on top in
    reference order ((x+y)+z); VectorE chunk-reduces PSUM halves.
  * y finales run in groups of YG batches so ScalarE can start squaring
    early; emission is interleaved per group so every engine queue stays
    in dependency order (S: squares only; V: reductions+finales+copies;
    PE: transposes + accumulate matmuls; G: indirect gathers).
  * DMA issue order: y0,y1,x0,z0,y2,y3,x1,z1,... so y planes (which gate
    the centroid) stream ahead of x/z.
"""

import numpy as np

import concourse.bacc as bacc
import concourse.bass as bass
import concourse.mybir as mybir
from concourse.masks import make_identity
from concourse.tile import TileContext

B = 64  # full batch
N_CORES = 8
BPC = B // N_CORES  # batches per core
N = 262144
P = 128
COLS = N // P  # 2048
K = 16  # chunks per partition row
J = 128  # columns per chunk
KBIG = 4096.0  # > P*K codes
ROWJ = 3 * N // J  # 6144 rows of 128 elems per batch (x,y,z planes)
H = COLS // 2  # half-plane columns (PSUM tile width)

YG = 2  # y-finale group size (must divide BPC)
NG = BPC // YG

F32 = mybir.dt.float32
U32 = mybir.dt.uint32
I32 = mybir.dt.int32
AX = mybir.AxisListType.X
OP = mybir.AluOpType
SQUARE = mybir.ActivationFunctionType.Square


def build_nc():
    nc = bacc.Bacc()
    xin = nc.dram_tensor("xyz", [BPC, 3, N], F32, kind="ExternalInput")
    out = nc.dram_tensor("idx", [1, 2 * BPC], I32, kind="ExternalOutput")

    xin_rows = xin.rearrange("b c (r j) -> (b c r) j", j=J)  # [BPC*3*2048, 128]
    xin_flat = xin.rearrange("b c n -> (b c n)")[:, None]  # [BPC*3*N, 1]

    with TileContext(nc) as tc:
        with (
            tc.tile_pool(name="consts", bufs=1) as consts,
            tc.tile_pool(name="ypool", bufs=BPC) as ypool,
            tc.tile_pool(name="xz", bufs=3) as xz,
            tc.tile_pool(name="sq", bufs=2) as sqp,
            tc.tile_pool(name="acc", bufs=1) as acc,
            tc.tile_pool(name="small", bufs=2) as small,
            tc.tile_pool(name="pd", bufs=2, space="PSUM") as pdp,
            tc.tile_pool(name="pt", bufs=3, space="PSUM") as ptp,
        ):
            # ---------- issue the first y DMAs before const setup ----------
            tys = [
                ypool.tile([P, COLS], F32, tag="ty", name=f"ty{b}")
                for b in range(BPC)
            ]
            nc.sync.dma_start(tys[0], xin[0, 1].rearrange("(p m) -> p m", p=P))
            nc.sync.dma_start(tys[1], xin[1, 1].rearrange("(p m) -> p m", p=P))

            # ---------- constants ----------
            ident = consts.tile([P, P], F32)
            make_identity(nc, ident)
            ones = consts.tile([1, P], F32)
            nc.vector.memset(ones, 1.0)

            # revk[p, b*K + k] = KBIG - (p*K + k)
            revk_i = consts.tile([P, P], I32)
            nc.gpsimd.iota(
                revk_i, pattern=[[0, BPC], [-1, K]], base=int(KBIG),
                channel_multiplier=-K,
            )
            revk = consts.tile([P, P], F32)
            nc.vector.tensor_copy(revk, revk_i)

            # revj[b, j] = J - j  (max picks smallest j)
            revj_i = consts.tile([BPC, J], I32)
            nc.gpsimd.iota(revj_i, pattern=[[-1, J]], base=J, channel_multiplier=0)
            revj = consts.tile([BPC, J], F32)
            nc.vector.tensor_copy(revj, revj_i)

            def iota_f32(g, base, mult, name):
                t_i = consts.tile([g, 1], I32, name=name + "i")
                nc.gpsimd.iota(
                    t_i, pattern=[[0, 1]], base=base, channel_multiplier=mult
                )
                t_f = consts.tile([g, 1], F32, name=name + "f")
                nc.vector.tensor_copy(t_f, t_i)
                return t_f

            # per-group consts (partitions 0..YG-1 hold batches g0..g0+YG-1):
            rowy_g = [
                iota_f32(YG, int(KBIG) + g0 * ROWJ + 2048, ROWJ, f"rowy{g0}")
                for g0 in range(0, BPC, YG)
            ]
            exbx_g = [
                iota_f32(YG, 524416 + g0 * 3 * N, 3 * N, f"exbx{g0}")
                for g0 in range(0, BPC, YG)
            ]
            exbz_g = [
                iota_f32(YG, 524416 + g0 * 3 * N + 2 * N, 3 * N, f"exbz{g0}")
                for g0 in range(0, BPC, YG)
            ]
            # dist-finale consts (all 8 batches): row = KBIG + b*ROWJ + c*2048
            rowc8 = [
                iota_f32(BPC, int(KBIG) + c * COLS, ROWJ, f"rowc8{c}")
                for c in range(3)
            ]

            # ---------- accumulators ----------
            Myall = acc.tile([P, P], F32)   # y chunk maxima, cols b*16+k
            Mdall = acc.tile([P, P], F32)   # dist chunk maxima
            nbx = acc.tile([P, BPC], F32)   # -cx broadcast bias columns
            nby = acc.tile([P, BPC], F32)
            nbz = acc.tile([P, BPC], F32)
            out_i = acc.tile([1, 2 * BPC], I32)

            def chunk_red(dst_cols, src_ap):
                nc.vector.tensor_reduce(
                    dst_cols,
                    src_ap.rearrange("p (k j) -> p k j", j=J),
                    axis=AX, op=OP.max,
                )

            def argmax_finale(Mall, b0, g, tag):
                """Global argmax for batches b0..b0+g-1 from chunk maxima.
                Returns (gm [g,1] global max, best [g,1] = KBIG - pk)."""
                mg = small.tile([P, g], F32, tag=f"mg{tag}", name=f"mg{tag}")
                nc.vector.tensor_reduce(
                    mg,
                    Mall[:, K * b0 : K * (b0 + g)].rearrange(
                        "p (g k) -> p g k", k=K
                    ),
                    axis=AX, op=OP.max,
                )
                pmg = ptp.tile([g, P], F32, tag="pt", name=f"pmg{tag}")
                nc.tensor.transpose(pmg, mg, ident)
                gm = small.tile([g, 1], F32, tag=f"gm{tag}", name=f"gm{tag}")
                nc.vector.tensor_reduce(gm, pmg, axis=AX, op=OP.max)
                # broadcast gm to all partitions: [128, g]
                pgt = ptp.tile([1, g], F32, tag="pt", name=f"pgt{tag}")
                nc.tensor.transpose(pgt, gm, ident[0:g, 0:g])
                gmr = small.tile([1, g], F32, tag=f"gmr{tag}", name=f"gmr{tag}")
                nc.vector.tensor_copy(gmr, pgt)
                pgb = ptp.tile([P, g], F32, tag="pt", name=f"pgb{tag}")
                nc.tensor.matmul(pgb, ones, gmr, start=True, stop=True)
                gmb = small.tile([P, g], F32, tag=f"gmb{tag}", name=f"gmb{tag}")
                nc.vector.tensor_copy(gmb, pgb)
                # cand = (M == gm) * revk ; max -> smallest pk code
                cand = small.tile(
                    [P, g * K], F32, tag=f"cand{tag}", name=f"cand{tag}"
                )
                for i in range(g):
                    nc.vector.scalar_tensor_tensor(
                        out=cand[:, K * i : K * (i + 1)],
                        in0=Mall[:, K * (b0 + i) : K * (b0 + i + 1)],
                        scalar=gmb[:, i : i + 1],
                        in1=revk[:, K * (b0 + i) : K * (b0 + i + 1)],
                        op0=OP.is_equal, op1=OP.mult,
                    )
                cred = small.tile([P, g], F32, tag=f"cred{tag}", name=f"cred{tag}")
                nc.vector.tensor_reduce(
                    cred, cand.rearrange("p (g k) -> p g k", k=K),
                    axis=AX, op=OP.max,
                )
                pcr = ptp.tile([g, P], F32, tag="pt", name=f"pcr{tag}")
                nc.tensor.transpose(pcr, cred, ident)
                best = small.tile([g, 1], F32, tag=f"best{tag}", name=f"best{tag}")
                nc.vector.tensor_reduce(best, pcr, axis=AX, op=OP.max)
                return gm, best

            def gather_rows(best, rowconst, g, tag):
                """row = rowconst - best; gather [g, J] window from HBM."""
                rowu = small.tile([g, 1], U32, tag=f"rowu{tag}", name=f"rowu{tag}")
                nc.vector.tensor_scalar(
                    out=rowu, in0=best, scalar1=-1.0, scalar2=rowconst,
                    op0=OP.mult, op1=OP.add,
                )
                win = small.tile([g, J], F32, tag=f"win{tag}", name=f"win{tag}")
                nc.gpsimd.indirect_dma_start(
                    out=win, out_offset=None, in_=xin_rows,
                    in_offset=bass.IndirectOffsetOnAxis(ap=rowu[0:g, 0:1], axis=0),
                )
                return win

            def window_find(win, gm, g, tag):
                """first column of win matching gm: returns wbest = J - j."""
                wc = small.tile([g, J], F32, tag=f"wc{tag}", name=f"wc{tag}")
                nc.vector.scalar_tensor_tensor(
                    out=wc, in0=win, scalar=gm, in1=revj[0:g, :],
                    op0=OP.is_equal, op1=OP.mult,
                )
                wbest = small.tile([g, 1], F32, tag=f"wb{tag}", name=f"wb{tag}")
                nc.vector.tensor_reduce(wbest, wc, axis=AX, op=OP.max)
                return wbest

            def emit_idx(best, wbest, g, out_cols, tag):
                """idx = 524416 - 128*best - wbest -> out_i (i32). Returns q."""
                q = small.tile([g, 1], F32, tag=f"q{tag}", name=f"q{tag}")
                nc.vector.scalar_tensor_tensor(
                    out=q, in0=best, scalar=-128.0, in1=wbest,
                    op0=OP.mult, op1=OP.subtract,
                )
                idxf = small.tile([g, 1], F32, tag=f"idxf{tag}", name=f"idxf{tag}")
                nc.vector.tensor_scalar(
                    out=idxf, in0=q, scalar1=1.0, scalar2=524416.0,
                    op0=OP.mult, op1=OP.add,
                )
                pidx = ptp.tile([1, g], F32, tag="pt", name=f"pidx{tag}")
                nc.tensor.transpose(pidx, idxf, ident[0:g, 0:g])
                nc.vector.tensor_copy(out_i[0:1, out_cols], pidx)
                return q

            def bias_cols(vals, g, b0, dst, tag):
                """dst[:, b0:b0+g] = broadcast of -vals ([g,1]) to all rows."""
                pv = ptp.tile([1, g], F32, tag="pt", name=f"pv{tag}")
                nc.tensor.transpose(pv, vals, ident[0:g, 0:g])
                nrow = small.tile([1, g], F32, tag=f"nrow{tag}", name=f"nrow{tag}")
                nc.vector.tensor_scalar(
                    out=nrow, in0=pv, scalar1=-1.0, scalar2=None, op0=OP.mult
                )
                pb = ptp.tile([P, g], F32, tag="pt", name=f"pb{tag}")
                nc.tensor.matmul(pb, ones, nrow, start=True, stop=True)
                nc.vector.tensor_copy(dst[:, b0 : b0 + g], pb)

            def y_group_finale(g0):
                g = YG
                gm, best = argmax_finale(Myall, g0, g, f"y{g0}")
                win = gather_rows(best, rowy_g[g0 // YG], g, f"y{g0}")
                wbest = window_find(win, gm, g, f"y{g0}")
                q = emit_idx(best, wbest, g, slice(g0, g0 + g), f"y{g0}")
                # centroid element gathers: off = q + (524416 + b*3N [+2N])
                offx = small.tile([g, 1], U32, tag=f"offx{g0}", name=f"offx{g0}")
                nc.vector.tensor_scalar(
                    out=offx, in0=q, scalar1=1.0, scalar2=exbx_g[g0 // YG],
                    op0=OP.mult, op1=OP.add,
                )
                offz = small.tile([g, 1], U32, tag=f"offz{g0}", name=f"offz{g0}")
                nc.vector.tensor_scalar(
                    out=offz, in0=q, scalar1=1.0, scalar2=exbz_g[g0 // YG],
                    op0=OP.mult, op1=OP.add,
                )
                cx = small.tile([g, 1], F32, tag=f"cx{g0}", name=f"cx{g0}")
                nc.gpsimd.indirect_dma_start(
                    out=cx, out_offset=None, in_=xin_flat,
                    in_offset=bass.IndirectOffsetOnAxis(ap=offx[0:g, 0:1], axis=0),
                )
                cz = small.tile([g, 1], F32, tag=f"cz{g0}", name=f"cz{g0}")
                nc.gpsimd.indirect_dma_start(
                    out=cz, out_offset=None, in_=xin_flat,
                    in_offset=bass.IndirectOffsetOnAxis(ap=offz[0:g, 0:1], axis=0),
                )
                bias_cols(cx, g, g0, nbx, f"bx{g0}")
                bias_cols(gm, g, g0, nby, f"by{g0}")  # cy == max y value
                bias_cols(cz, g, g0, nbz, f"bz{g0}")

            txs, tzs = {}, {}

            def issue_xz(b):
                tx = xz.tile([P, COLS], F32, tag="tx", name=f"tx{b}")
                nc.sync.dma_start(tx, xin[b, 0].rearrange("(p m) -> p m", p=P))
                tz = xz.tile([P, COLS], F32, tag="tz", name=f"tz{b}")
                nc.sync.dma_start(tz, xin[b, 2].rearrange("(p m) -> p m", p=P))
                txs[b], tzs[b] = tx, tz

            def phase_b(b):
                """squares + PSUM accumulate + chunk reduce for one batch.
                S: ACT x-halves straight into PSUM, y/z squares to SBUF.
                PE: += y half, += z half per PSUM half (order (x+y)+z)."""
                pds = []
                for h in range(2):
                    pd = pdp.tile([P, H], F32, tag="pd", name=f"pd{b}_{h}")
                    pds.append(pd)
                    nc.scalar.activation(
                        pd, txs[b][:, h * H : (h + 1) * H], SQUARE,
                        bias=nbx[:, b : b + 1],
                    )
                sqy = sqp.tile([P, COLS], F32, tag="sqy", name=f"sqy{b}")
                nc.scalar.activation(sqy, tys[b], SQUARE, bias=nby[:, b : b + 1])
                sqz = sqp.tile([P, COLS], F32, tag="sqz", name=f"sqz{b}")
                nc.scalar.activation(sqz, tzs[b], SQUARE, bias=nbz[:, b : b + 1])
                for h in range(2):
                    pd = pds[h]
                    for ci, sq in ((1, sqy), (2, sqz)):
                        for c0 in range(0, H, 512):
                            nc.tensor.matmul(
                                pd[:, c0 : c0 + 512],
                                ident,
                                sq[:, h * H + c0 : h * H + c0 + 512],
                                start=False,
                                stop=(ci == 2 and c0 + 512 >= H),
                                skip_group_check=True,
                            )
                    chunk_red(
                        Mdall[:, K * b + h * (K // 2) : K * b + (h + 1) * (K // 2)],
                        pd,
                    )

            # ---------- interleaved emission ----------
            # iteration i: y DMAs for group i (already issued for i=0),
            # y-reduces, y-finale(i), xz DMA issues, phase_b for group i-1.
            for i in range(NG):
                b0 = YG * i
                if i > 0:
                    for b in (b0, b0 + 1):
                        nc.sync.dma_start(
                            tys[b], xin[b, 1].rearrange("(p m) -> p m", p=P)
                        )
                chunk_red(Myall[:, K * b0 : K * (b0 + 1)], tys[b0])
                chunk_red(Myall[:, K * (b0 + 1) : K * (b0 + 2)], tys[b0 + 1])
                y_group_finale(b0)
                issue_xz(2 * i)
                issue_xz(2 * i + 1)
                if i > 0:
                    phase_b(YG * (i - 1))
                    phase_b(YG * (i - 1) + 1)
            phase_b(BPC - 2)
            phase_b(BPC - 1)

            # ---------- dist finale (batched over all 8) ----------
            gm_d, best_d = argmax_finale(Mdall, 0, BPC, "d")
            # negated centroids per batch in [8,1] layout: diag of nb* via
            # mask-mult + ADD-reduce (values are negative; max would be wrong)
            negc8 = []
            for name, nb in (("x", nbx), ("y", nby), ("z", nbz)):
                dtmp = small.tile(
                    [BPC, BPC], F32, tag=f"dg{name}", name=f"dg{name}"
                )
                nc.vector.tensor_tensor(
                    dtmp, nb[0:BPC, :], ident[0:BPC, 0:BPC], op=OP.mult
                )
                dneg = small.tile([BPC, 1], F32, tag=f"dn{name}", name=f"dn{name}")
                nc.vector.tensor_reduce(dneg, dtmp, axis=AX, op=OP.add)
                negc8.append(dneg)
            wins = [gather_rows(best_d, rowc8[c], BPC, f"d{c}") for c in range(3)]
            wsq = []
            for c in range(3):
                s = small.tile([BPC, J], F32, tag=f"wsq{c}", name=f"wsq{c}")
                nc.scalar.activation(s, wins[c], SQUARE, bias=negc8[c])
                wsq.append(s)
            wd1 = small.tile([BPC, J], F32, tag="wd1", name="wd1")
            nc.vector.tensor_add(wd1, wsq[0], wsq[1])
            wd2 = small.tile([BPC, J], F32, tag="wd2", name="wd2")
            nc.vector.tensor_add(wd2, wd1, wsq[2])
            wbest_d = window_find(wd2, gm_d, BPC, "d")
            emit_idx(best_d, wbest_d, BPC, slice(BPC, 2 * BPC), "d")

            nc.sync.dma_start(out[:, :], out_i[:, :])

    nc.compile()
    return nc


_NC_CACHE = None


def _get_nc():
    global _NC_CACHE
    if _NC_CACHE is None:
        _NC_CACHE = build_nc()
    return _NC_CACHE


def kernel(xyz: np.ndarray) -> np.ndarray:
    from concourse.bass_utils import run_bass_kernel_spmd

    assert xyz.shape == (1, B, 3, N), xyz.shape
    xyz = np.ascontiguousarray(xyz, dtype=np.float32)
    nc = _get_nc()
    in_maps = [
        {"xyz": np.ascontiguousarray(xyz[0, k * BPC : (k + 1) * BPC])}
        for k in range(N_CORES)
    ]
    res = run_bass_kernel_spmd(nc, in_maps, core_ids=list(range(N_CORES)))
    # out layout per core: [1, 16] = [idx0 x8 | idx1 x8]
    outs = [res.results[k]["idx"].reshape(2, BPC).T for k in range(N_CORES)]
    return np.concatenate(outs, axis=0).astype(np.int64)
